# revision 32
# baseline (speedup 1.0000x reference)
"""RBF kernel ridge regression inference on 8 Trainium2 NeuronCores.

out[q] = sum_t exp(-gamma * ||X[q] - T[t]||^2) * coef[t],
with w[t] = exp(-g*y2[t]) * coef[t] so out[q] = sum_t exp(2g*dot - g*x2) * w.

All three compute engines are balanced at ~2.1us per [128q, 2048t] tile:

- TensorE: fp8 DoubleRow GEMM in [q_part, t_free] layout (256 MMs instead of
  512; warm MMs issue every ~216ns when consecutive MMs hit the same psum
  bank, hence the s-outer/j-inner order).
- ScalarE: one Exp per 4-bank psum group ([128, 2048] ACTIVATE, per-partition
  bias -g*x2[q]).
- Reduction over t, split to whichever engine has slack:
  * 24 tiles on VectorE: S += sum_t et[q,t] * w[t] via scalar_tensor_tensor
    free-axis accumulate (w as a broadcast bf16 row).
  * 8 sign-pure tiles on ScalarE for free via ACTIVATE's accum_out: the host
    sign-sorts train points (negatives first, flipped so negatives are the
    majority), and |w[t]| is folded INTO the exp argument as a t-varying row
    r[t] = (ln|w[t]| - B)/2g added to the dot by 4 tile-position-packed K=1
    matmuls (concurrent in distinct 32-row strips, ~0.6us per tile); the
    per-partition ACT bias carries -g*x2[q] + B.  accum then yields
    +-sum_t et*|w| directly; signs are applied in the final combine.

Startup hides the ~22us of replicated-input DMA: non-critical streams get
tile_wait_until floors so the first tile's operands get full bandwidth, and
36 throwaway DoubleRow matmuls keep the PE busy so the HAM clock gate is at
8/8 when real work starts; the exp table is preloaded the same way.

Queries are sharded across the 8 cores; train side is replicated.  Host
precomputes the tiny O(N*d) prep: permutation, transposes, fp8/bf16 casts,
row norms, ln|w| folding (0.05% of FLOPs; the GEMM+exp+reduce run on device).
"""

import numpy as np
import ml_dtypes

GAMMA = 1.0
N_QUERY, N_TRAIN, D = 8192, 8192, 512
N_CORES = 8
P = 128
QPC = N_QUERY // N_CORES  # 1024 queries per core
KS = D // P               # 4 contraction subtiles (d = ks*128 + p)
NQC = QPC // P            # 8 query chunks of 128
GT = 2048                 # train columns per psum group (4 banks)
NG = N_TRAIN // GT        # 4 groups
ST = 512                  # train cols per matmul (one psum bank)
NS = GT // ST             # 4 subtiles per group
# (g, c) tiles whose reduction runs on ScalarE via ACT accum_out.  Host
# sign-sorts train points (negatives first, majority negative after flip),
# so g=0/g=1 tiles are all-negative and g=3 tiles all-positive; g=2 holds
# the sign boundary and must reduce on DVE (sign-row multiply).  The g=3
# entries cover the last-processed tiles so the kernel doesn't end on a
# trailing DVE op.
SCC_NEG = ((0, 2), (0, 6))
SCC_POS = ((3, 4), (3, 5), (3, 6), (3, 7))
SCC = SCC_NEG + SCC_POS

_CACHE = {}


def _build_program():
    from contextlib import ExitStack

    import concourse.bass as bass
    import concourse.mybir as mybir
    import concourse.tile as tile
    from concourse import bacc

    f32 = mybir.dt.float32
    bf16 = mybir.dt.bfloat16
    f8 = mybir.dt.float8e4
    AF = mybir.ActivationFunctionType
    MUL = mybir.AluOpType.mult
    DR = mybir.MatmulPerfMode.DoubleRow

    nc = bacc.Bacc(
        "TRN2", target_bir_lowering=False, debug=False, num_devices=N_CORES
    )

    tt_d = nc.dram_tensor("tt_fp8", [D, N_TRAIN], f8, kind="ExternalInput").ap()
    x_d = nc.dram_tensor("x_fp8", [D, QPC], f8, kind="ExternalInput").ap()
    wb_d = nc.dram_tensor("wb_bf16", [P, N_TRAIN], bf16, kind="ExternalInput").ap()
    r8_d = nc.dram_tensor("r8_fp8", [1, N_TRAIN], f8, kind="ExternalInput").ap()
    x2_d = nc.dram_tensor("x2n_f32", [P, 2 * NQC], f32, kind="ExternalInput").ap()
    out_d = nc.dram_tensor("out", [QPC], f32, kind="ExternalOutput").ap()

    with tile.TileContext(nc) as tc, ExitStack() as ctx:
        res = ctx.enter_context(tc.tile_pool(name="res", bufs=1))
        etp = ctx.enter_context(tc.tile_pool(name="etp", bufs=4))
        scrp = ctx.enter_context(tc.tile_pool(name="scrp", bufs=4))
        psq = ctx.enter_context(tc.tile_pool(name="psq", bufs=2, space="PSUM"))

        x_sb = res.tile([P, KS, QPC], f8, tag="x")
        x2_sb = res.tile([P, 2 * NQC], f32, tag="x2")
        wb_sb = res.tile([P, N_TRAIN], bf16, tag="wb")
        r8_sb = res.tile([P, N_TRAIN], f8, tag="r8")
        tt_sb = res.tile([P, KS, N_TRAIN], f8, tag="tt")
        Scol = res.tile([P, NQC * NG], f32, tag="Scol")
        Acol = res.tile([P, max(1, len(SCC))], f32, tag="Acol")
        out_sb = res.tile([P, NQC], f32, tag="out")
        warm_sb = res.tile([P, 2, 128], f8, tag="warm")
        eights = res.tile([P, P], f8, tag="eights")
        ghost = res.tile([P, GT], bf16, tag="ghost")

        # loads split across the two hwdge queues (Sync + ScalarE); the
        # non-critical streams get wait_until floors so the first tiles'
        # operands (~1.6MB) get the full DMA bandwidth instead of sharing
        # it with 5MB of later-needed data
        nc.scalar.dma_start(x2_sb[:], x2_d[:])
        nc.scalar.dma_start(
            x_sb[:, :, 0:P], x_d[:, 0:P].rearrange("(k p) q -> p k q", k=KS)
        )
        with tc.tile_wait_until(0.0125):
            nc.scalar.dma_start(
                x_sb[:, :, P:], x_d[:, P:].rearrange("(k p) q -> p k q", k=KS)
            )
        for b in range(NS):
            nc.scalar.dma_start(r8_sb[32 * b : 32 * b + 1, :], r8_d[:])
        nc.sync.dma_start(
            tt_sb[:, 0:2, 0:GT],
            tt_d[0 : 2 * P, 0:GT].rearrange("(k p) t -> p k t", k=2),
        )
        nc.sync.dma_start(
            tt_sb[:, 2:4, 0:GT],
            tt_d[2 * P :, 0:GT].rearrange("(k p) t -> p k t", k=2),
        )
        with tc.tile_wait_until(0.014):
            nc.sync.dma_start(wb_sb[:, 0:GT], wb_d[:, 0:GT])
        with tc.tile_wait_until(0.014):
            nc.sync.dma_start(
                tt_sb[:, :, GT : 2 * GT],
                tt_d[:, GT : 2 * GT].rearrange("(k p) t -> p k t", k=KS),
            )
        with tc.tile_wait_until(0.018):
            nc.sync.dma_start(wb_sb[:, GT : 2 * GT], wb_d[:, GT : 2 * GT])
        with tc.tile_wait_until(0.020):
            nc.sync.dma_start(
                tt_sb[:, :, 2 * GT : 3 * GT],
                tt_d[:, 2 * GT : 3 * GT].rearrange("(k p) t -> p k t", k=KS),
            )
        with tc.tile_wait_until(0.028):
            nc.sync.dma_start(
                tt_sb[:, :, 3 * GT :],
                tt_d[:, 3 * GT :].rearrange("(k p) t -> p k t", k=KS),
            )
            nc.sync.dma_start(wb_sb[:, 2 * GT :], wb_d[:, 2 * GT :])

        nc.vector.memset(warm_sb[:], 0)
        nc.vector.memset(eights[:], 8.0)
        nc.vector.memset(Scol[:], 0.0)
        # preload the exp table while DMAs stream (first real ACT would
        # otherwise pay the ~2.7us ACT_TABLE_LOAD on the critical path)
        warm_act = res.tile([P, 1], bf16, tag="wact")
        nc.scalar.activation(warm_act[:], warm_sb[:, 0, 0:1], AF.Exp, scale=1.0)

        # HAM warmup: keep the PE busy while the first train tiles stream in,
        # so the clock gate is at 8/8 when the real matmuls start.  Results
        # land in the first psum tile's banks and are discarded by the real
        # accumulation groups' start=True.
        ps0 = psq.tile([P, GT], f32, tag="ps")
        for r in range(36):
            nc.tensor.matmul(
                ps0[:, 0:128],
                warm_sb[:],
                warm_sb[:],
                start=True,
                stop=True,
                perf_mode=DR,
                skip_group_check=True,
            )

        first = True
        for g in range(NG):
            for c in range(NQC):
                ps = ps0 if first else psq.tile([P, GT], f32, tag="ps")
                first = False
                sc_tile = (g, c) in SCC
                for s in range(NS):
                    for j in range(KS // 2):
                        nc.tensor.matmul(
                            ps[:, s * ST : (s + 1) * ST],
                            x_sb[:, 2 * j : 2 * j + 2, c * P : (c + 1) * P],
                            tt_sb[:, 2 * j : 2 * j + 2, g * GT + s * ST : g * GT + (s + 1) * ST],
                            start=(j == 0),
                            stop=(j == KS // 2 - 1) and not sc_tile,
                            perf_mode=DR,
                        )
                if sc_tile:
                    # ScalarE-reduced tile: add the t-varying ln|w[t]| - B row
                    # via 4 concurrently-packed K=1 matmuls (distinct 32-row
                    # strips, distinct psum banks): ps[:, bank s] += 8*(r/8)
                    for s in range(NS):
                        nc.tensor.matmul(
                            ps[:, s * ST : (s + 1) * ST],
                            eights[32 * s : 32 * s + 1, :],
                            r8_sb[32 * s : 32 * s + 1, g * GT + s * ST : g * GT + (s + 1) * ST],
                            start=False,
                            stop=True,
                            tile_position=(32 * s, 0),
                        )
                    # sign-pure tile: ScalarE reduces it for free via accum
                    nc.scalar.activation(
                        ghost[:],
                        ps[:],
                        AF.Exp,
                        bias=x2_sb[:, NQC + c : NQC + c + 1],
                        scale=2.0 * GAMMA,
                        accum_out=Acol[:, SCC.index((g, c)) : SCC.index((g, c)) + 1],
                    )
                else:
                    et = etp.tile([P, GT], bf16, tag="et")
                    nc.scalar.activation(
                        et[:], ps[:], AF.Exp, bias=x2_sb[:, c : c + 1], scale=2.0 * GAMMA
                    )
                    scr = scrp.tile([P, GT], bf16, tag="scr")
                    nc.vector.scalar_tensor_tensor(
                        scr[:],
                        et[:],
                        1.0,
                        wb_sb[:, g * GT : (g + 1) * GT],
                        MUL,
                        MUL,
                        accum_out=Scol[:, c * NG + g : c * NG + g + 1],
                    )
        for c in range(NQC):
            nc.vector.tensor_reduce(
                out_sb[:, c : c + 1],
                Scol[:, c * NG : (c + 1) * NG],
                axis=mybir.AxisListType.X,
                op=mybir.AluOpType.add,
            )
            for (g2, c2) in SCC_NEG:
                if c2 == c:
                    # all-negative-coef tile: subtract its ScalarE accum
                    nc.vector.tensor_tensor(
                        out_sb[:, c : c + 1],
                        out_sb[:, c : c + 1],
                        Acol[:, SCC.index((g2, c2)) : SCC.index((g2, c2)) + 1],
                        mybir.AluOpType.subtract,
                    )
            for (g2, c2) in SCC_POS:
                if c2 == c:
                    nc.vector.tensor_tensor(
                        out_sb[:, c : c + 1],
                        out_sb[:, c : c + 1],
                        Acol[:, SCC.index((g2, c2)) : SCC.index((g2, c2)) + 1],
                        mybir.AluOpType.add,
                    )
        # p-major out layout: per-partition contiguous 32B runs instead of
        # 1024 scattered 4B descriptors; kernel() un-permutes on host
        nc.sync.dma_start(out_d.rearrange("(p c) -> p c", p=P), out_sb[:])

    nc.compile()
    return nc


def _get_program():
    if "nc" not in _CACHE:
        _CACHE["nc"] = _build_program()
    return _CACHE["nc"]


def make_in_maps(X, train_X, dual_coef):
    bf = ml_dtypes.bfloat16
    f8 = ml_dtypes.float8_e4m3

    X = np.asarray(X, dtype=np.float32)
    train_X = np.asarray(train_X, dtype=np.float32)
    dual_coef = np.asarray(dual_coef, dtype=np.float32)

    # flip so negative coefs are the majority (>= 4096 >= GT); the host
    # negates the final output back.  Then sort negatives first so the
    # g=0 tiles (first GT columns) are sign-pure for the ScalarE reduction.
    flip = (dual_coef < 0).sum() < N_TRAIN // 2
    coef = -dual_coef if flip else dual_coef
    perm = np.concatenate([np.where(coef < 0)[0], np.where(coef >= 0)[0]])
    coef = coef[perm]
    train_s = train_X[perm]

    ttq = np.ascontiguousarray(train_s.T).astype(f8)          # [D, N_TRAIN]
    y2 = np.einsum("td,td->t", train_s, train_s)              # [N_TRAIN]
    lnw = -GAMMA * y2 + np.log(np.maximum(np.abs(coef), 1e-30))
    B = float(np.mean(lnw))
    # ACT applies scale=2g to the whole psum (dot + r-row), so pre-divide
    # r by 2g; /8 pairs with the constant-8 stationary of the bias matmuls
    r8 = ((lnw - B) / (2.0 * GAMMA) / 8.0).astype(f8).reshape(1, N_TRAIN)
    w = (np.exp(-GAMMA * y2) * coef).astype(bf)               # DVE-tile weights
    wbb = np.ascontiguousarray(np.broadcast_to(w[None, :], (P, N_TRAIN)))
    x2 = np.einsum("qd,qd->q", X, X)                          # [N_QUERY]
    XT = np.ascontiguousarray(X.T)                            # [D, N_QUERY]

    in_maps = []
    for c in range(N_CORES):
        xs = np.ascontiguousarray(XT[:, c * QPC : (c + 1) * QPC]).astype(f8)
        x2n = -GAMMA * x2[c * QPC : (c + 1) * QPC]
        x2c = np.ascontiguousarray(
            np.concatenate(
                [x2n.reshape(NQC, P).T, (x2n + B).reshape(NQC, P).T], axis=1
            ).astype(np.float32)
        )
        in_maps.append(
            {
                "tt_fp8": ttq,
                "x_fp8": xs,
                "wb_bf16": wbb,
                "r8_fp8": r8,
                "x2n_f32": x2c,
            }
        )
    return in_maps, flip


def _get_callable():
    """Cached (fn, in_names, out_names, out_avals, zero_outs, mesh) for the
    sharded 8-core NEFF execution."""
    if "call" in _CACHE:
        return _CACHE["call"]

    import jax
    from jax.sharding import Mesh, PartitionSpec
    from jax.experimental.shard_map import shard_map

    import concourse.mybir as mybir
    from concourse import bass2jax
    from concourse.bass2jax import install_neuronx_cc_hook

    install_neuronx_cc_hook()
    nc = _get_program()

    partition_name = (
        nc.partition_id_tensor.name if nc.partition_id_tensor else None
    )
    in_names, out_names, out_avals, zero_outs = [], [], [], []
    for alloc in nc.m.functions[0].allocations:
        if not isinstance(alloc, mybir.MemoryLocationSet):
            continue
        if alloc.kind not in ("ExternalInput", "ExternalOutput"):
            continue
        name = alloc.memorylocations[0].name
        if alloc.kind == "ExternalInput":
            if name != partition_name:
                in_names.append(name)
        else:
            out_names.append(name)
            shape = tuple(alloc.tensor_shape)
            dtype = mybir.dt.np(alloc.dtype)
            out_avals.append(jax.core.ShapedArray(shape, dtype))
            zero_outs.append(np.zeros(shape, dtype))
    all_in_names = in_names + out_names
    if partition_name is not None:
        all_in_names = all_in_names + [partition_name]

    def _body(*args):
        operands = list(args)
        if partition_name is not None:
            operands.append(bass2jax.partition_id_tensor())
        outs = bass2jax._bass_exec_p.bind(
            *operands,
            out_avals=tuple(out_avals),
            in_names=tuple(all_in_names),
            out_names=tuple(out_names),
            lowering_input_output_aliases=(),
            sim_require_finite=True,
            sim_require_nnan=True,
            nc=nc,
        )
        return tuple(outs)

    devices = jax.devices()[:N_CORES]
    mesh = Mesh(np.asarray(devices), ("core",))
    n_all = len(in_names) + len(out_names)
    fn = jax.jit(
        shard_map(
            _body,
            mesh=mesh,
            in_specs=(PartitionSpec("core"),) * n_all,
            out_specs=(PartitionSpec("core"),) * len(out_names),
            check_rep=False,
        ),
        keep_unused=True,
    )
    _CACHE["call"] = (fn, in_names, out_names, out_avals, zero_outs, mesh)
    return _CACHE["call"]


def concat_inputs(in_maps):
    fn, in_names, out_names, out_avals, zero_outs, mesh = _get_callable()
    concat_in = [
        np.concatenate([np.asarray(m[name]) for m in in_maps], axis=0)
        for name in in_names
    ]
    concat_zeros = [
        np.zeros((N_CORES * z.shape[0], *z.shape[1:]), z.dtype) for z in zero_outs
    ]
    return concat_in + concat_zeros


def kernel(X, train_X, dual_coef):
    X = np.asarray(X, dtype=np.float32)
    train_X = np.asarray(train_X, dtype=np.float32)
    dual_coef = np.asarray(dual_coef, dtype=np.float32)

    fn, in_names, out_names, out_avals, zero_outs, mesh = _get_callable()
    in_maps, flip = make_in_maps(X, train_X, dual_coef)
    args = concat_inputs(in_maps)
    outs = fn(*args)
    out = np.asarray(outs[0]).reshape(N_CORES, P, NQC)
    # device wrote p-major ([p, c] with q = c*128 + p); un-permute per core
    out = out.transpose(0, 2, 1).reshape(-1)
    if flip:
        out = -out
    return np.ascontiguousarray(out).astype(np.float32)
